# revision 47
# baseline (speedup 1.0000x reference)
"""DeepSeek-MLA forward kernel for 8 Trainium2 NeuronCores (Bass/Tile).

Sharding: core c -> batch b = c // 4, head-group g = c % 4 (4 of 16 heads).
Each core computes its batch's down-projections (replicated x4 within the
batch group), its 4 heads' attention, and a partial output projection
y_part = out_heads_local @ w_o_local.  The host sums the 4 partials per
batch and stacks the 2 batches.

v10 (this file): restructured for engine overlap -- measured 222.5us on HW
(2 runs: 222470/222812ns; baseline was 344.7us), rel err 4.69e-3:
 - down-proj weight DMAs chunked and interleaved with the first x block so
   the first matmul starts ~2us earlier,
 - softmax accumulators evacuated from PSUM to SBUF with one fast DVE copy
   so the bank recycles in ~0.7us instead of being held through the whole
   ln/exp/broadcast/mul normalize chain (~4.5us),
 - rope block software-pipelined TWO s-blocks behind, attention av matmuls
   pipelined one i-tile behind their scores, output projection deferred one
   j-block -- all so the in-order PE queue never head-of-line blocks on a
   cross-engine chain (norm, exp, or softmax-eviction),
 - single ACT table set (exp/ln/copy steered into
   natural_log_exp_and_others; was 26 table reloads = 33us),
 - phases 1+2 fused per s-block with the rope block software-pipelined one
   iteration behind, so the in-order PE queue never head-of-line blocks on
   the RMS-norm chain and the PE stays HAM-warm into attention,
 - critical-path-ordered input DMAs (down-proj weights + first x block
   first; small statics slotted behind),
 - head-pair-batched softmax exp ([128,1024] per i-tile, diagonal tiles as
   trimmed 2-page 3D APs) straight off double-buffered score PSUM,
 - per-j-block PSUM pool swap: 4-buffer oacc during attention, then the
   output-projection pool, so P4 matmuls/DMA interleave per j-block,
 - PSUM->SBUF evictions split across ACT (c/v copies) and DVE,
 - y written back in bf16 (host accumulates partials in f32).

Note: nc.vector.reciprocal_approx_fast mis-evaluates in this stack (reads
stale/garbage PSUM; confirmed in CoreSim and HW) -- softmax 1/den stays on
ACT as exp(-ln(den)).

Precision: matmul inputs bf16, PSUM accumulation fp32, softmax stats fp32.
"""

import os
import sys

import numpy as np

for _p in ("/opt/trn_rl_repo", "/root/.axon_site/_ro/trn_rl_repo"):
    if os.path.isdir(_p) and _p not in sys.path:
        sys.path.insert(0, _p)

import concourse.bass as bass
import concourse.mybir as mybir
import concourse.tile as tile
from concourse import bacc

B, S, D, H, DN, DR, R = 2, 2048, 2048, 16, 32, 32, 128
HD = DN + DR  # 64
EPS = 1e-5
NCORES = 8
NH = 4          # heads per core
SB = 512        # s-block (psum bank width in f32)
NSB = S // SB   # 4
ST = 128        # s-tile
NST = S // ST   # 16
KC = 128        # contraction chunk
NKC = D // KC   # 16
VW = HD + 1     # v columns incl. ones column (65)
F32 = mybir.dt.float32
BF16 = mybir.dt.bfloat16

_ACT_PATCHED = False


def _patch_act_tables():
    """Steer Exp and Ln to the one table set containing both, so the
    act-table-load pass emits a single ACT_TABLE_LOAD instead of
    thrashing between exp_and_others and natural_log (26 loads, ~33us)."""
    global _ACT_PATCHED
    if _ACT_PATCHED:
        return
    import concourse.bacc as bacc_mod
    from concourse.hw_specs import get_activation_tables as _gat

    AF = mybir.ActivationFunctionType

    def _gat_steered(arch):
        tabs = _gat(arch)
        for name, fns in tabs.items():
            if name != "natural_log_exp_and_others":
                fns.discard(AF.Exp)
                fns.discard(AF.Ln)
                fns.discard(AF.Copy)
                fns.discard(AF.Identity)
        return tabs

    bacc_mod.get_activation_tables = _gat_steered
    _ACT_PATCHED = True


def _build_nc_fast(debug_dump=False):
    """Optimized causal (no-mask) builder."""
    _patch_act_tables()
    nc = bacc.Bacc("TRN2", target_bir_lowering=False, debug=False,
                   num_devices=NCORES)

    xT = nc.dram_tensor("xT", [D, S], BF16, kind="ExternalInput").ap()
    wkv = nc.dram_tensor("wkv", [KC, D], BF16, kind="ExternalInput").ap()
    wq = nc.dram_tensor("wq", [KC, D], BF16, kind="ExternalInput").ap()
    kb = nc.dram_tensor("kb", [R, 2 * KC], BF16, kind="ExternalInput").ap()
    ksh = nc.dram_tensor("ksh", [R, 2 * KC], BF16, kind="ExternalInput").ap()
    qb = nc.dram_tensor("qb", [R, 2 * KC], BF16, kind="ExternalInput").ap()
    qsh = nc.dram_tensor("qsh", [R, 2 * KC], BF16, kind="ExternalInput").ap()
    uv = nc.dram_tensor("uv", [R, NH * HD], BF16, kind="ExternalInput").ap()
    wo = nc.dram_tensor("wo", [KC, 2 * D], BF16, kind="ExternalInput").ap()
    cosP = nc.dram_tensor("cosP", [128, S], BF16, kind="ExternalInput").ap()
    sinP = nc.dram_tensor("sinP", [128, S], BF16, kind="ExternalInput").ap()
    y = nc.dram_tensor("y", [S, D], BF16, kind="ExternalOutput").ap()
    dbg = {}
    if debug_dump:
        for nm, shp in (("d_ckvT", [R, S]), ("d_cqT", [R, S]),
                        ("d_kT01", [128, S]), ("d_kT23", [128, S]),
                        ("d_qT01", [128, S]), ("d_qT23", [128, S]),
                        ("d_v", [128, NST * NH * VW]),
                        ("d_outT01", [128, S]), ("d_outT23", [128, S]),
                        ("d_et", [128, 2 * SB])):
            dbg[nm] = nc.dram_tensor(nm, shp, BF16,
                                     kind="ExternalOutput").ap()

    AF = mybir.ActivationFunctionType
    ALU = mybir.AluOpType

    with tile.TileContext(nc) as tc:
        from contextlib import ExitStack
        with ExitStack() as ctx:
            stat = ctx.enter_context(tc.tile_pool(name="static", bufs=1))
            # persistent SBUF tensors
            ckvT = stat.tile([R, S], BF16, name="ckvT")
            cqT = stat.tile([R, S], BF16, name="cqT")
            kT01 = stat.tile([128, S], BF16, name="kT01")
            kT23 = stat.tile([128, S], BF16, name="kT23")
            qT01 = stat.tile([128, S], BF16, name="qT01")
            qT23 = stat.tile([128, S], BF16, name="qT23")
            v_sb = stat.tile([128, NST * NH * VW], BF16, name="v_sb")
            outT01 = stat.tile([128, S], BF16, name="outT01")
            outT23 = stat.tile([128, S], BF16, name="outT23")
            kb_sb = stat.tile([R, 2 * KC], BF16, name="kb_sb")
            ksh_sb = stat.tile([R, 2 * KC], BF16, name="ksh_sb")
            qb_sb = stat.tile([R, 2 * KC], BF16, name="qb_sb")
            qsh_sb = stat.tile([R, 2 * KC], BF16, name="qsh_sb")
            uv_sb = stat.tile([R, NH * HD], BF16, name="uv_sb")
            wo_sb = stat.tile([KC, 2 * D], BF16, name="wo_sb")
            eps_sb = stat.tile([128, 1], F32, name="eps_sb")

            cosP_sb = stat.tile([128, S], BF16, name="cosP_sb")
            sinP_sb = stat.tile([128, S], BF16, name="sinP_sb")
            tri_sb = stat.tile([128, 128], BF16, name="tri_sb")
            onesf_sb = stat.tile([128, 64], F32, name="onesf_sb")
            nc.gpsimd.memset(onesf_sb[:], 1.0)
            nc.gpsimd.memset(eps_sb[:], EPS)
            # tri[p, f] = 1.0 if p <= f else 0.0 (keep-lower-triangle gate
            # for diagonal score strips in k-major layout)
            nc.gpsimd.memset(tri_sb[:], 1.0)
            nc.gpsimd.affine_select(
                out=tri_sb[:], in_=tri_sb[:], compare_op=ALU.is_ge,
                fill=0.0, base=0, channel_multiplier=-1, pattern=[[1, 128]])
            # ones column of v (col 64 of each 65-wide block)
            v_blocks = v_sb.rearrange("p (t h w) -> p t h w", t=NST, h=NH)
            nc.vector.tensor_copy(
                v_blocks[:, :, :, HD:VW],
                onesf_sb.rearrange("p (t h w) -> p t h w", t=NST, h=NH))

            xTr = xT.rearrange("(a p) s -> p a s", p=KC)   # [128, NKC, S]

            # ------- Phase 1+2 fused per s-block: down-proj + RMS norm +
            # ------- up-projections + rope + v  (keeps the PE warm) -------
            with tc.tile_pool(name="p1w", bufs=1) as p1w, \
                 tc.tile_pool(name="p1x", bufs=2) as p1x, \
                 tc.tile_pool(name="p1n", bufs=6) as p1n, \
                 tc.tile_pool(name="p2tmp", bufs=6) as p2tmp, \
                 tc.tile_pool(name="p1ps", bufs=2, space="PSUM") as p1ps, \
                 tc.tile_pool(name="p2ps", bufs=2, space="PSUM") as p2ps, \
                 tc.tile_pool(name="p2vps", bufs=2, space="PSUM") as p2vps:
                wkv_sb = p1w.tile([KC, D], BF16, name="wkv_sb")
                wq_sb = p1w.tile([KC, D], BF16, name="wq_sb")
                # critical-path DMA order: first chunk of the down-proj
                # weights, then x groups interleaved with the remaining
                # weight chunks (the k-loop only needs chunk g at step 4g),
                # then the small static weights.
                nc.sync.dma_start(wkv_sb[:, 0:512], wkv[:, 0:512])
                nc.sync.dma_start(wq_sb[:, 0:512], wq[:, 0:512])
                from concourse import bass_isa

                # rope for block sb runs one iteration later (software
                # pipeline) so its PE matmuls never head-of-line block the
                # next block's down-projection matmuls on the norm chain.
                def rope_block(sb):
                    sl = slice(sb * SB, (sb + 1) * SB)
                    # pair tensors: rows [he_nope|he_rope|ho_nope|ho_rope];
                    # cosP rows are 1.0 (sinP rows 0.0) on nope rows so one
                    # fused 3-op rope pass covers nope+rope together.
                    for cT, wb, wsh, dst in (
                            (ckvT, kb_sb, ksh_sb, (kT01, kT23)),
                            (cqT, qb_sb, qsh_sb, (qT01, qT23))):
                        for p in range(2):
                            ps2 = p2ps.tile([128, 2 * SB], F32, name="ps2",
                                            tag="p2")
                            nc.tensor.matmul(ps2[:, 0:SB],
                                             wb[:, p * KC:(p + 1) * KC],
                                             cT[:, sl])
                            nc.tensor.matmul(ps2[:, SB:2 * SB],
                                             wsh[:, p * KC:(p + 1) * KC],
                                             cT[:, sl])
                            t1 = p2tmp.tile([128, SB], BF16, name="t1", tag="t")
                            t2 = p2tmp.tile([128, SB], BF16, name="t2", tag="t")
                            nc.vector.tensor_mul(t1[:], ps2[:, 0:SB],
                                                 cosP_sb[:, sl])
                            nc.vector.tensor_mul(t2[:], ps2[:, SB:2 * SB],
                                                 sinP_sb[:, sl])
                            nc.vector.tensor_add(dst[p][:, sl], t1[:], t2[:])
                    # v tiles for this s-block (4 t-tiles)
                    for t in range(4 * sb, 4 * sb + 4):
                        vps = p2vps.tile([128, NH * HD], F32, name="vps",
                                         tag="v")
                        nc.tensor.matmul(vps[:], ckvT[:, t * ST:(t + 1) * ST],
                                         uv_sb[:])
                        nc.scalar.copy(
                            v_blocks[:, t, :, 0:HD],
                            vps.rearrange("p (h d) -> p h d", h=NH))

                for sb in range(NSB):
                    sl = slice(sb * SB, (sb + 1) * SB)
                    xt = p1x.tile([128, NKC * SB], BF16, name="xt", tag="xt")
                    xtv = xt.rearrange("p (a s) -> p a s", a=NKC)
                    for g in range(4):
                        nc.sync.dma_start(xtv[:, 4 * g:4 * g + 4, :],
                                          xTr[:, 4 * g:4 * g + 4, sl])
                        if sb == 0 and g < 3:
                            csl = slice((g + 1) * 512, (g + 2) * 512)
                            nc.sync.dma_start(wkv_sb[:, csl], wkv[:, csl])
                            nc.sync.dma_start(wq_sb[:, csl], wq[:, csl])
                    # deferred static loads, slotted behind the x blocks
                    nc.sync.dma_start(cosP_sb[:, sl], cosP[:, sl])
                    nc.sync.dma_start(sinP_sb[:, sl], sinP[:, sl])
                    if sb == 0:
                        nc.sync.dma_start(kb_sb[:], kb)
                        nc.sync.dma_start(ksh_sb[:], ksh)
                        nc.sync.dma_start(qb_sb[:], qb)
                        nc.sync.dma_start(qsh_sb[:], qsh)
                        nc.sync.dma_start(uv_sb[:], uv)
                    elif sb == 1:
                        nc.sync.dma_start(wo_sb[:], wo)
                    cps = {nm: p1ps.tile([128, SB], F32, name=f"cps_{nm}",
                                         tag="cps") for nm in ("kv", "q")}
                    for k in range(NKC):
                        for nm, wsb in (("kv", wkv_sb), ("q", wq_sb)):
                            nc.tensor.matmul(
                                cps[nm][:],
                                wsb[:, k * KC:(k + 1) * KC],
                                xtv[:, k, :],
                                start=(k == 0), stop=(k == NKC - 1))
                    # both PSUM->SBUF copies first: frees the cps banks for
                    # the next block's matmuls as early as possible
                    for nm, cT in (("kv", ckvT), ("q", cqT)):
                        nc.scalar.copy(cT[:, sl], cps[nm][:])
                    for nm, cT in (("kv", ckvT), ("q", cqT)):
                        sqt = p1n.tile([128, SB], F32, name="sqt", tag="sqt")
                        rst = p1n.tile([128, SB], F32, name="rst", tag="rst")
                        nc.vector.tensor_mul(sqt[:], cT[:, sl], cT[:, sl])
                        nc.gpsimd.partition_all_reduce(
                            rst[:], sqt[:], channels=128,
                            reduce_op=bass_isa.ReduceOp.add)
                        # rstd = (sum/R + eps)^-1/2 = exp(-0.5 * ln(...))
                        nc.scalar.activation(rst[:], rst[:], AF.Ln,
                                             bias=eps_sb[:], scale=1.0 / R)
                        nc.scalar.activation(rst[:], rst[:], AF.Exp,
                                             scale=-0.5)
                        nc.vector.tensor_mul(cT[:, sl], cT[:, sl], rst[:])
                    if sb > 1:
                        rope_block(sb - 2)
                rope_block(NSB - 2)
                rope_block(NSB - 1)

            if debug_dump:
                nc.sync.dma_start(dbg["d_ckvT"], ckvT[:])
                nc.sync.dma_start(dbg["d_cqT"], cqT[:])
                nc.sync.dma_start(dbg["d_kT01"], kT01[:])
                nc.sync.dma_start(dbg["d_kT23"], kT23[:])
                nc.sync.dma_start(dbg["d_qT01"], qT01[:])
                nc.sync.dma_start(dbg["d_qT23"], qT23[:])
                nc.sync.dma_start(dbg["d_v"], v_sb[:])

            # ------- Phase 3+4: attention + interleaved output proj ------
            with tc.tile_pool(name="p3e", bufs=4) as p3e, \
                 tc.tile_pool(name="p3rc", bufs=4) as p3rc, \
                 tc.tile_pool(name="p3rb", bufs=4) as p3rb, \
                 tc.tile_pool(name="p4y", bufs=4) as p4y, \
                 tc.tile_pool(name="p3sc", bufs=2, space="PSUM") as p3sc, \
                 tc.tile_pool(name="p3oa", bufs=2, space="PSUM") as p3oa, \
                 tc.tile_pool(name="p4ps", bufs=2, space="PSUM") as p4ps:
                # output projection deferred one j-block so its matmuls
                # never wait on the softmax eviction chain
                def p4_block(j, pps, py):
                    for t in range(4 * j, 4 * j + 4):
                        for db in range(NSB):
                            dsl = slice(db * SB, (db + 1) * SB)
                            yps = pps.tile([128, SB], F32, name="yps",
                                           tag="yp")
                            for c, oT in ((0, outT01), (1, outT23)):
                                nc.tensor.matmul(
                                    yps[:],
                                    oT[:, t * ST:(t + 1) * ST],
                                    wo_sb[:, c * D + db * SB:
                                          c * D + (db + 1) * SB],
                                    start=(c == 0), stop=(c == 1))
                            ysb = py.tile([128, SB], BF16, name="ysb",
                                          tag="y")
                            nc.vector.tensor_copy(ysb[:], yps[:])
                            nc.sync.dma_start(
                                y[t * ST:(t + 1) * ST, dsl], ysb[:])

                for j in range(NSB):
                    for hp in range(2):      # head pairs (0,1) and (2,3)
                        kTp = kT01 if hp == 0 else kT23
                        qTp = qT01 if hp == 0 else qT23
                        oa = [p3oa.tile([VW, SB], F32, name=f"oa{j}_{hp}{u}",
                                        tag="oa") for u in range(2)]
                        ktiles = list(range(4 * j + 4))
                        pend = None   # (i, q0, et) awaiting its av matmuls
                        for i in ktiles + [None]:
                            if i is not None:
                                q0 = 128 * (i - 4 * j) if i >= 4 * j else 0
                                scp = p3sc.tile([128, 2 * SB], F32,
                                                name="scp", tag="sc")
                                for u in range(2):
                                    hs = slice(u * 64, u * 64 + 64)
                                    nc.tensor.matmul(
                                        scp[:, u * SB + q0:(u + 1) * SB],
                                        kTp[hs, i * 128:(i + 1) * 128],
                                        qTp[hs, j * SB + q0:(j + 1) * SB])
                                et = p3e.tile([128, 2 * SB], BF16, name="et",
                                              tag="et")
                                if q0 == 0:
                                    nc.scalar.activation(et[:], scp[:],
                                                         AF.Exp, scale=0.125)
                                else:
                                    sc3 = scp.rearrange("p (g s) -> p g s",
                                                        g=2)
                                    et3 = et.rearrange("p (g s) -> p g s",
                                                       g=2)
                                    nc.scalar.activation(
                                        et3[:, :, q0:SB], sc3[:, :, q0:SB],
                                        AF.Exp, scale=0.125)
                                if i >= 4 * j:
                                    for u in range(2):
                                        nc.vector.tensor_mul(
                                            et[:, u * SB + q0:
                                               u * SB + q0 + 128],
                                            et[:, u * SB + q0:
                                               u * SB + q0 + 128],
                                            tri_sb[:])
                                if debug_dump and j == 0 and hp == 0 and i == 0:
                                    nc.sync.dma_start(dbg["d_et"], et[:])
                            # av matmuls for the previous i-tile, emitted
                            # after the next scores so the PE never idles
                            # waiting on the exp chain
                            if pend is not None:
                                pi, pq0, pet = pend
                                for u in range(2):
                                    h = 2 * hp + u
                                    nc.tensor.matmul(
                                        oa[u][:, pq0:SB],
                                        v_sb[:, pi * (NH * VW) + h * VW:
                                             pi * (NH * VW) + (h + 1) * VW],
                                        pet[:, u * SB + pq0:(u + 1) * SB],
                                        start=(pi == 0),
                                        stop=(pi == ktiles[-1]))
                            pend = (i, q0, et) if i is not None else None
                        for u in range(2):
                            h = 2 * hp + u
                            # evacuate oacc to SBUF in one fast copy so the
                            # PSUM bank recycles immediately; the normalize
                            # chain then runs entirely off SBUF
                            oc = p3rb.tile([VW, SB], F32, name="oc", tag="oc")
                            nc.vector.tensor_copy(oc[:], oa[u][:])
                            rc = p3rc.tile([1, SB], F32, name="rc", tag="rc")
                            nc.scalar.activation(rc[:], oc[HD:VW, :], AF.Ln)
                            nc.scalar.activation(rc[:], rc[:], AF.Exp,
                                                 scale=-1.0)
                            rb = p3rb.tile([HD, SB], F32, name="rb", tag="rb")
                            nc.gpsimd.partition_broadcast(rb[:], rc[:])
                            dstT = (outT01 if h < 2 else outT23)[
                                (h % 2) * HD:(h % 2 + 1) * HD,
                                j * SB:(j + 1) * SB]
                            nc.vector.tensor_mul(dstT, oc[0:HD, :], rb[:])
                    if j in (1, 2):
                        p4_block(j - 1, p4ps, p4y)
                if debug_dump:
                    nc.sync.dma_start(dbg["d_outT01"], outT01[:])
                    nc.sync.dma_start(dbg["d_outT23"], outT23[:])

            # last two output-projection blocks run post-attention: give
            # them a deep PSUM pool so bank recycling never waits on the
            # evacuation casts queued behind the eviction chain
            with tc.tile_pool(name="p4yb", bufs=8) as p4yb, \
                 tc.tile_pool(name="p4big", bufs=6, space="PSUM") as p4big:
                p4_block(2, p4big, p4yb)
                p4_block(3, p4big, p4yb)

    nc.finalize()
    return nc


def _build_nc_generic(causal: bool, use_mask: bool):
    """Baseline builder (kept as fallback for non-causal / masked inputs)."""
    nc = bacc.Bacc("TRN2", target_bir_lowering=False, debug=False,
                   num_devices=NCORES)

    xT = nc.dram_tensor("xT", [D, S], BF16, kind="ExternalInput").ap()
    wkv = nc.dram_tensor("wkv", [KC, D], BF16, kind="ExternalInput").ap()
    wq = nc.dram_tensor("wq", [KC, D], BF16, kind="ExternalInput").ap()
    kb = nc.dram_tensor("kb", [R, 2 * KC], BF16, kind="ExternalInput").ap()
    ksh = nc.dram_tensor("ksh", [R, 2 * KC], BF16, kind="ExternalInput").ap()
    qb = nc.dram_tensor("qb", [R, 2 * KC], BF16, kind="ExternalInput").ap()
    qsh = nc.dram_tensor("qsh", [R, 2 * KC], BF16, kind="ExternalInput").ap()
    uv = nc.dram_tensor("uv", [R, NH * HD], BF16, kind="ExternalInput").ap()
    wo = nc.dram_tensor("wo", [KC, 2 * D], BF16, kind="ExternalInput").ap()
    cosP = nc.dram_tensor("cosP", [128, S], F32, kind="ExternalInput").ap()
    sinP = nc.dram_tensor("sinP", [128, S], F32, kind="ExternalInput").ap()
    maskT = None
    if use_mask:
        maskT = nc.dram_tensor("maskT", [S, S], F32, kind="ExternalInput").ap()
    y = nc.dram_tensor("y", [S, D], F32, kind="ExternalOutput").ap()

    AF = mybir.ActivationFunctionType
    ALU = mybir.AluOpType

    with tile.TileContext(nc) as tc:
        from contextlib import ExitStack
        with ExitStack() as ctx:
            stat = ctx.enter_context(tc.tile_pool(name="static", bufs=1))
            ckvT = stat.tile([R, S], BF16, name="ckvT")
            cqT = stat.tile([R, S], BF16, name="cqT")
            kT01 = stat.tile([128, S], BF16, name="kT01")
            kT23 = stat.tile([128, S], BF16, name="kT23")
            qT01 = stat.tile([128, S], BF16, name="qT01")
            qT23 = stat.tile([128, S], BF16, name="qT23")
            v_sb = stat.tile([128, NST * NH * VW], BF16, name="v_sb")
            outT01 = stat.tile([128, S], BF16, name="outT01")
            outT23 = stat.tile([128, S], BF16, name="outT23")
            kb_sb = stat.tile([R, 2 * KC], BF16, name="kb_sb")
            ksh_sb = stat.tile([R, 2 * KC], BF16, name="ksh_sb")
            qb_sb = stat.tile([R, 2 * KC], BF16, name="qb_sb")
            qsh_sb = stat.tile([R, 2 * KC], BF16, name="qsh_sb")
            uv_sb = stat.tile([R, NH * HD], BF16, name="uv_sb")
            wo_sb = stat.tile([KC, 2 * D], BF16, name="wo_sb")
            eps_sb = stat.tile([128, 1], F32, name="eps_sb")

            nc.sync.dma_start(kb_sb[:], kb)
            nc.sync.dma_start(ksh_sb[:], ksh)
            nc.sync.dma_start(qb_sb[:], qb)
            nc.sync.dma_start(qsh_sb[:], qsh)
            nc.sync.dma_start(uv_sb[:], uv)
            nc.sync.dma_start(wo_sb[:], wo)
            tri_sb = stat.tile([128, 128], BF16, name="tri_sb")
            onesf_sb = stat.tile([128, 64], F32, name="onesf_sb")
            nc.gpsimd.memset(onesf_sb[:], 1.0)
            nc.gpsimd.memset(eps_sb[:], EPS)
            nc.gpsimd.memset(tri_sb[:], 1.0)
            nc.gpsimd.affine_select(
                out=tri_sb[:], in_=tri_sb[:], compare_op=ALU.is_ge,
                fill=0.0, base=0, channel_multiplier=-1, pattern=[[1, 128]])
            v_blocks = v_sb.rearrange("p (t h w) -> p t h w", t=NST, h=NH)
            nc.vector.tensor_copy(
                v_blocks[:, :, :, HD:VW],
                onesf_sb.rearrange("p (t h w) -> p t h w", t=NST, h=NH))

            from concourse import bass_isa
            with tc.tile_pool(name="p1w", bufs=1) as p1w, \
                 tc.tile_pool(name="p1n", bufs=8) as p1n, \
                 tc.tile_pool(name="p1x", bufs=8) as p1x, \
                 tc.tile_pool(name="p1ps", bufs=6, space="PSUM") as p1ps:
                wkv_sb = p1w.tile([KC, D], BF16, name="wkv_sb")
                wq_sb = p1w.tile([KC, D], BF16, name="wq_sb")
                nc.sync.dma_start(wkv_sb[:], wkv)
                nc.sync.dma_start(wq_sb[:], wq)

                for sb in range(NSB):
                    sl = slice(sb * SB, (sb + 1) * SB)
                    cps = {nm: p1ps.tile([128, SB], F32, name=f"cps_{nm}",
                                         tag="cps") for nm in ("kv", "q")}
                    for k in range(NKC):
                        xt = p1x.tile([128, SB], BF16, name="xt", tag="xt")
                        nc.sync.dma_start(xt[:], xT[k * KC:(k + 1) * KC, sl])
                        for nm, wsb in (("kv", wkv_sb), ("q", wq_sb)):
                            nc.tensor.matmul(
                                cps[nm][:],
                                wsb[:, k * KC:(k + 1) * KC],
                                xt[:],
                                start=(k == 0), stop=(k == NKC - 1))
                    for nm, cT in (("kv", ckvT), ("q", cqT)):
                        sqt = p1n.tile([128, SB], F32, name="sqt", tag="sqt")
                        rst = p1n.tile([128, SB], F32, name="rst", tag="sqt")
                        nc.vector.tensor_copy(cT[:, sl], cps[nm][:])
                        nc.vector.tensor_mul(sqt[:], cT[:, sl], cT[:, sl])
                        nc.gpsimd.partition_all_reduce(
                            rst[:], sqt[:], channels=128,
                            reduce_op=bass_isa.ReduceOp.add)
                        nc.scalar.activation(rst[:], rst[:], AF.Ln,
                                             bias=eps_sb[:], scale=1.0 / R)
                        nc.scalar.activation(rst[:], rst[:], AF.Exp,
                                             scale=-0.5)
                        nc.vector.tensor_mul(cT[:, sl], cT[:, sl], rst[:])

            with tc.tile_pool(name="p2t", bufs=1) as p2t, \
                 tc.tile_pool(name="p2tmp", bufs=6) as p2tmp, \
                 tc.tile_pool(name="p2ps", bufs=6, space="PSUM") as p2ps, \
                 tc.tile_pool(name="p2vps", bufs=2, space="PSUM") as p2vps:
                cosP_sb = p2t.tile([128, S], F32, name="cosP_sb")
                sinP_sb = p2t.tile([128, S], F32, name="sinP_sb")
                nc.sync.dma_start(cosP_sb[:], cosP)
                nc.sync.dma_start(sinP_sb[:], sinP)

                for sb in range(NSB):
                    sl = slice(sb * SB, (sb + 1) * SB)
                    for cT, wb, wsh, dst in (
                            (ckvT, kb_sb, ksh_sb, (kT01, kT23)),
                            (cqT, qb_sb, qsh_sb, (qT01, qT23))):
                        for p in range(2):
                            pb = p2ps.tile([128, SB], F32, name="pb", tag="p2")
                            psh = p2ps.tile([128, SB], F32, name="psh", tag="p2")
                            nc.tensor.matmul(pb[:], wb[:, p * KC:(p + 1) * KC],
                                             cT[:, sl])
                            nc.tensor.matmul(psh[:], wsh[:, p * KC:(p + 1) * KC],
                                             cT[:, sl])
                            t1 = p2tmp.tile([128, SB], F32, name="t1", tag="t")
                            t2 = p2tmp.tile([128, SB], F32, name="t2", tag="t")
                            nc.vector.tensor_mul(t1[:], pb[:], cosP_sb[:, sl])
                            nc.vector.tensor_mul(t2[:], psh[:], sinP_sb[:, sl])
                            nc.vector.tensor_add(dst[p][:, sl], t1[:], t2[:])

                for t in range(NST):
                    vps = p2vps.tile([128, NH * HD], F32, name="vps", tag="v")
                    nc.tensor.matmul(vps[:], (ckvT[:, t * ST:(t + 1) * ST]),
                                     (uv_sb[:]))
                    dst = v_blocks[:, t, :, 0:HD]
                    src = vps.rearrange("p (h d) -> p h d", h=NH)
                    nc.vector.tensor_copy(dst, src)

            with tc.tile_pool(name="p3e", bufs=8) as p3e, \
                 tc.tile_pool(name="p3m", bufs=3) as p3m, \
                 tc.tile_pool(name="p3rc", bufs=8) as p3rc, \
                 tc.tile_pool(name="p3rb", bufs=6) as p3rb, \
                 tc.tile_pool(name="p3sc", bufs=4, space="PSUM") as p3sc, \
                 tc.tile_pool(name="p3oa", bufs=4, space="PSUM") as p3oa:
                for j in range(NSB):
                    ktiles = list(range(4 * j + 4)) if causal else list(range(NST))
                    oacc = [p3oa.tile([VW, SB], F32, name=f"oa{j}_{h}", tag="oa")
                            for h in range(NH)]
                    for i in ktiles:
                        q0 = 128 * (i - 4 * j) if (causal and i >= 4 * j) else 0
                        qsl = slice(q0, SB)
                        mt = None
                        if use_mask:
                            mt = p3m.tile([128, SB], F32, name="mt", tag="mt")
                            nc.sync.dma_start(
                                mt[:], maskT[i * 128:(i + 1) * 128,
                                             j * SB:(j + 1) * SB])
                        for h in range(NH):
                            kTp = kT01 if h < 2 else kT23
                            qTp = qT01 if h < 2 else qT23
                            hs = slice((h % 2) * 64, (h % 2) * 64 + 64)
                            sc = p3sc.tile([128, SB], F32, name="sc", tag="sc")
                            nc.tensor.matmul(
                                sc[:, qsl],
                                kTp[hs, i * 128:(i + 1) * 128],
                                qTp[hs, j * SB + q0:(j + 1) * SB])
                            if use_mask:
                                nc.vector.tensor_add(sc[:, qsl], sc[:, qsl],
                                                     mt[:, qsl])
                            et = p3e.tile([128, SB], BF16, name="et", tag="e")
                            nc.scalar.activation(et[:, qsl], sc[:, qsl],
                                                 AF.Exp, scale=0.125)
                            if causal and i >= 4 * j:
                                nc.vector.tensor_mul(et[:, q0:q0 + 128],
                                                     et[:, q0:q0 + 128],
                                                     tri_sb[:])
                            nc.tensor.matmul(
                                oacc[h][:, qsl],
                                (v_sb[:, i * (NH * VW) + h * VW:
                                        i * (NH * VW) + (h + 1) * VW]),
                                (et[:, qsl]),
                                start=(i == ktiles[0]), stop=(i == ktiles[-1]))
                    for h in range(NH):
                        rc = p3rc.tile([1, SB], F32, name="rc", tag="rc")
                        nc.scalar.activation(rc[:], oacc[h][HD:VW, :], AF.Ln)
                        nc.scalar.activation(rc[:], rc[:], AF.Exp, scale=-1.0)
                        rb = p3rb.tile([HD, SB], F32, name="rb", tag="rb")
                        nc.gpsimd.partition_broadcast(rb[:], rc[:])
                        dst = (outT01 if h < 2 else outT23)[
                            (h % 2) * HD:(h % 2 + 1) * HD,
                            j * SB:(j + 1) * SB]
                        nc.vector.tensor_mul(dst, oacc[h][0:HD, :], rb[:])

            with tc.tile_pool(name="p4y", bufs=4) as p4y, \
                 tc.tile_pool(name="p4ps", bufs=2, space="PSUM") as p4ps:
                for t in range(NST):
                    yp = p4ps.tile([128, S], F32, name="yp", tag="yp")
                    for db in range(NSB):
                        dsl = slice(db * SB, (db + 1) * SB)
                        for c, oT in ((0, outT01), (1, outT23)):
                            nc.tensor.matmul(
                                yp[:, dsl],
                                (oT[:, t * ST:(t + 1) * ST]),
                                (wo_sb[:, c * D + db * SB:c * D + (db + 1) * SB]),
                                start=(c == 0), stop=(c == 1))
                    ysb = p4y.tile([128, S], F32, name="ysb", tag="y")
                    nc.vector.tensor_copy(ysb[:], yp[:])
                    nc.sync.dma_start(y[t * ST:(t + 1) * ST, :], ysb[:])

    nc.finalize()
    return nc


_NC_CACHE = {}


def _get_nc(causal, use_mask):
    key = (causal, use_mask)
    if key not in _NC_CACHE:
        if causal and not use_mask:
            _NC_CACHE[key] = _build_nc_fast()
        else:
            _NC_CACHE[key] = _build_nc_generic(causal, use_mask)
    return _NC_CACHE[key]


def _prep_inputs(x, cos, sin, mask, w_kv_down, kv_norm_w, w_uk, w_ur, w_uv,
                 w_q_down, q_norm_w, w_uq, w_qr, w_o, use_mask):
    """Build the 8 per-core input maps (host-side shard + fold)."""
    f = np.float32
    x = np.asarray(x, f)
    cos = np.asarray(cos, f)
    sin = np.asarray(sin, f)
    w_kv_down = np.asarray(w_kv_down, f)
    w_q_down = np.asarray(w_q_down, f)
    kv_norm_w = np.asarray(kv_norm_w, f)
    q_norm_w = np.asarray(q_norm_w, f)
    w_uk_e = np.asarray(w_uk, f) * kv_norm_w[:, None]
    w_ur_e = np.asarray(w_ur, f) * kv_norm_w[:, None]
    w_uv_e = np.asarray(w_uv, f) * kv_norm_w[:, None]
    w_uq_e = np.asarray(w_uq, f) * q_norm_w[:, None]
    w_qr_e = np.asarray(w_qr, f) * q_norm_w[:, None]
    w_o = np.asarray(w_o, f)

    # shared rearrangements
    wkv = np.ascontiguousarray(
        w_kv_down.reshape(NKC, KC, R).transpose(1, 0, 2).reshape(KC, D))
    wq = np.ascontiguousarray(
        w_q_down.reshape(NKC, KC, R).transpose(1, 0, 2).reshape(KC, D))
    cosT = np.ascontiguousarray(cos.T)                 # [32, S]
    sinT = np.ascontiguousarray(sin.T)
    sinSg = np.concatenate([-sinT[:DR // 2], sinT[DR // 2:]], axis=0)
    one32 = np.ones((DR, S), np.float32)
    zero32 = np.zeros((DR, S), np.float32)
    # pair-tensor rope tables: nope rows pass through (cos=1, sin=0)
    cosPt = np.ascontiguousarray(
        np.concatenate([one32, cosT, one32, cosT], axis=0))
    sinPt = np.ascontiguousarray(
        np.concatenate([zero32, sinSg, zero32, sinSg], axis=0))
    # rope shift permutation within each head's 32 cols
    perm = np.concatenate([np.arange(16, 32), np.arange(0, 16)])

    import ml_dtypes as _md
    xTb = [np.ascontiguousarray(x[b].T).astype(_md.bfloat16) for b in range(B)]
    maskT8 = None
    if use_mask:
        m = np.asarray(mask, f).reshape(S, S)
        maskT8 = np.ascontiguousarray(m.T) * 8.0

    in_maps = []
    z32 = np.zeros((R, DN), np.float32)
    for core in range(NCORES):
        b, g = core // 4, core % 4
        cs = slice(g * NH * DN, (g + 1) * NH * DN)      # 128-wide col slice
        vs = slice(g * NH * HD, (g + 1) * NH * HD)      # 256-wide
        uk_l = w_uk_e[:, cs].reshape(R, NH, DN)
        ur_l = w_ur_e[:, cs].reshape(R, NH, DR)
        urs_l = ur_l[:, :, perm]
        uq_l = w_uq_e[:, cs].reshape(R, NH, DN)
        qr_l = w_qr_e[:, cs].reshape(R, NH, DR)
        qrs_l = qr_l[:, :, perm]
        # pair layout: [he_nope | he_rope | ho_nope | ho_rope] per 128 cols
        def pair(nope, rope):
            cols = []
            for h in range(NH):
                cols += [nope[:, h], rope[:, h]]
            return np.ascontiguousarray(np.concatenate(cols, axis=1))
        def pair_sh(sh):
            cols = []
            for h in range(NH):
                cols += [z32, sh[:, h]]
            return np.ascontiguousarray(np.concatenate(cols, axis=1))
        wo_loc = w_o[g * NH * HD:(g + 1) * NH * HD]     # [256, D]
        wo_r = np.ascontiguousarray(
            wo_loc.reshape(2, KC, D).transpose(1, 0, 2).reshape(KC, 2 * D)
        ).astype(_md.bfloat16)
        cosW = cosPt if use_mask else cosPt.astype(_md.bfloat16)
        sinW = sinPt if use_mask else sinPt.astype(_md.bfloat16)
        m_ = {
            "xT": xTb[b],
            "wkv": wkv.astype(_md.bfloat16), "wq": wq.astype(_md.bfloat16),
            "kb": pair(uk_l, ur_l).astype(_md.bfloat16),
            "ksh": pair_sh(urs_l).astype(_md.bfloat16),
            "qb": pair(uq_l, qr_l).astype(_md.bfloat16),
            "qsh": pair_sh(qrs_l).astype(_md.bfloat16),
            "uv": np.ascontiguousarray(w_uv_e[:, vs]).astype(_md.bfloat16),
            "wo": wo_r,
            "cosP": cosW, "sinP": sinW,
        }
        if use_mask:
            m_["maskT"] = maskT8
        in_maps.append(m_)
    return in_maps


def _classify_mask(mask):
    m = np.asarray(mask, np.float32).reshape(S, S)
    if not np.any(m):
        return False, False          # dense, no mask
    causal_ref = np.where(
        np.tril(np.ones((S, S), dtype=bool)), np.float32(0.0),
        np.float32(-1e9))
    if np.array_equal(m, causal_ref):
        return True, False           # structural causal
    return False, True               # generic additive mask


LAST_RESULTS = None


def kernel(**inputs):
    global LAST_RESULTS
    from concourse.bass_utils import run_bass_kernel_spmd
    causal, use_mask = _classify_mask(inputs["mask"])
    nc = _get_nc(causal, use_mask)
    in_maps = _prep_inputs(
        inputs["x"], inputs["cos"], inputs["sin"], inputs["mask"],
        inputs["w_kv_down"], inputs["kv_norm_w"], inputs["w_uk"],
        inputs["w_ur"], inputs["w_uv"], inputs["w_q_down"],
        inputs["q_norm_w"], inputs["w_uq"], inputs["w_qr"], inputs["w_o"],
        use_mask)
    res = run_bass_kernel_spmd(nc, in_maps, list(range(NCORES)))
    LAST_RESULTS = res
    parts = [res.results[c]["y"] for c in range(NCORES)]
    out = np.empty((B, S, D), np.float32)
    for b in range(B):
        out[b] = (parts[4 * b].astype(np.float32)
                  + parts[4 * b + 1].astype(np.float32)
                  + parts[4 * b + 2].astype(np.float32)
                  + parts[4 * b + 3].astype(np.float32))
    return out


# revision 49
# speedup vs baseline: 1.0343x; 1.0343x over previous
"""DeepSeek-MLA forward kernel for 8 Trainium2 NeuronCores (Bass/Tile).

Sharding: core c -> batch b = c // 4, head-group g = c % 4 (4 of 16 heads).
Each core computes its batch's down-projections (replicated x4 within the
batch group), its 4 heads' attention, and a partial output projection
y_part = out_heads_local @ w_o_local.  The host sums the 4 partials per
batch and stacks the 2 batches.

v10 (this file): restructured for engine overlap -- measured 222.5us on HW
(2 runs: 222470/222812ns; baseline was 344.7us), rel err 4.69e-3:
 - down-proj weight DMAs chunked and interleaved with the first x block so
   the first matmul starts ~2us earlier,
 - softmax accumulators evacuated from PSUM to SBUF with one fast DVE copy
   so the bank recycles in ~0.7us instead of being held through the whole
   ln/exp/broadcast/mul normalize chain (~4.5us),
 - rope block software-pipelined TWO s-blocks behind, attention av matmuls
   pipelined one i-tile behind their scores, output projection deferred one
   j-block -- all so the in-order PE queue never head-of-line blocks on a
   cross-engine chain (norm, exp, or softmax-eviction),
 - single ACT table set (exp/ln/copy steered into
   natural_log_exp_and_others; was 26 table reloads = 33us),
 - phases 1+2 fused per s-block with the rope block software-pipelined one
   iteration behind, so the in-order PE queue never head-of-line blocks on
   the RMS-norm chain and the PE stays HAM-warm into attention,
 - critical-path-ordered input DMAs (down-proj weights + first x block
   first; small statics slotted behind),
 - head-pair-batched softmax exp ([128,1024] per i-tile, diagonal tiles as
   trimmed 2-page 3D APs) straight off double-buffered score PSUM,
 - per-j-block PSUM pool swap: 4-buffer oacc during attention, then the
   output-projection pool, so P4 matmuls/DMA interleave per j-block,
 - PSUM->SBUF evictions split across ACT (c/v copies) and DVE,
 - y written back in bf16 (host accumulates partials in f32).

Note: nc.vector.reciprocal_approx_fast mis-evaluates in this stack (reads
stale/garbage PSUM; confirmed in CoreSim and HW) -- softmax 1/den stays on
ACT as exp(-ln(den)).

Precision: matmul inputs bf16, PSUM accumulation fp32, softmax stats fp32.
"""

import os
import sys

import numpy as np

for _p in ("/opt/trn_rl_repo", "/root/.axon_site/_ro/trn_rl_repo"):
    if os.path.isdir(_p) and _p not in sys.path:
        sys.path.insert(0, _p)

import concourse.bass as bass
import concourse.mybir as mybir
import concourse.tile as tile
from concourse import bacc

B, S, D, H, DN, DR, R = 2, 2048, 2048, 16, 32, 32, 128
HD = DN + DR  # 64
EPS = 1e-5
NCORES = 8
NH = 4          # heads per core
SB = 512        # s-block (psum bank width in f32)
NSB = S // SB   # 4
ST = 128        # s-tile
NST = S // ST   # 16
KC = 128        # contraction chunk
NKC = D // KC   # 16
VW = HD + 1     # v columns incl. ones column (65)
F32 = mybir.dt.float32
BF16 = mybir.dt.bfloat16

_ACT_PATCHED = False


def _patch_act_tables():
    """Steer Exp and Ln to the one table set containing both, so the
    act-table-load pass emits a single ACT_TABLE_LOAD instead of
    thrashing between exp_and_others and natural_log (26 loads, ~33us)."""
    global _ACT_PATCHED
    if _ACT_PATCHED:
        return
    import concourse.bacc as bacc_mod
    from concourse.hw_specs import get_activation_tables as _gat

    AF = mybir.ActivationFunctionType

    def _gat_steered(arch):
        tabs = _gat(arch)
        for name, fns in tabs.items():
            if name != "natural_log_exp_and_others":
                fns.discard(AF.Exp)
                fns.discard(AF.Ln)
                fns.discard(AF.Copy)
                fns.discard(AF.Identity)
        return tabs

    bacc_mod.get_activation_tables = _gat_steered
    _ACT_PATCHED = True


def _build_nc_fast(debug_dump=False):
    """Optimized causal (no-mask) builder."""
    _patch_act_tables()
    nc = bacc.Bacc("TRN2", target_bir_lowering=False, debug=False,
                   num_devices=NCORES)

    xT = nc.dram_tensor("xT", [D, S], BF16, kind="ExternalInput").ap()
    wkv = nc.dram_tensor("wkv", [KC, D], BF16, kind="ExternalInput").ap()
    wq = nc.dram_tensor("wq", [KC, D], BF16, kind="ExternalInput").ap()
    kb = nc.dram_tensor("kb", [R, 2 * KC], BF16, kind="ExternalInput").ap()
    ksh = nc.dram_tensor("ksh", [R, 2 * KC], BF16, kind="ExternalInput").ap()
    qb = nc.dram_tensor("qb", [R, 2 * KC], BF16, kind="ExternalInput").ap()
    qsh = nc.dram_tensor("qsh", [R, 2 * KC], BF16, kind="ExternalInput").ap()
    uv = nc.dram_tensor("uv", [R, NH * HD], BF16, kind="ExternalInput").ap()
    wo = nc.dram_tensor("wo", [KC, 2 * D], BF16, kind="ExternalInput").ap()
    cosP = nc.dram_tensor("cosP", [128, S], BF16, kind="ExternalInput").ap()
    sinP = nc.dram_tensor("sinP", [128, S], BF16, kind="ExternalInput").ap()
    y = nc.dram_tensor("y", [S, D], BF16, kind="ExternalOutput").ap()
    dbg = {}
    if debug_dump:
        for nm, shp in (("d_ckvT", [R, S]), ("d_cqT", [R, S]),
                        ("d_kT01", [128, S]), ("d_kT23", [128, S]),
                        ("d_qT01", [128, S]), ("d_qT23", [128, S]),
                        ("d_v", [128, NST * NH * VW]),
                        ("d_outT01", [128, S]), ("d_outT23", [128, S]),
                        ("d_et", [128, 2 * SB])):
            dbg[nm] = nc.dram_tensor(nm, shp, BF16,
                                     kind="ExternalOutput").ap()

    AF = mybir.ActivationFunctionType
    ALU = mybir.AluOpType

    with tile.TileContext(nc) as tc:
        from contextlib import ExitStack
        with ExitStack() as ctx:
            stat = ctx.enter_context(tc.tile_pool(name="static", bufs=1))
            # persistent SBUF tensors
            ckvT = stat.tile([R, S], BF16, name="ckvT")
            cqT = stat.tile([R, S], BF16, name="cqT")
            kT01 = stat.tile([128, S], BF16, name="kT01")
            kT23 = stat.tile([128, S], BF16, name="kT23")
            qT01 = stat.tile([128, S], BF16, name="qT01")
            qT23 = stat.tile([128, S], BF16, name="qT23")
            v_sb = stat.tile([128, NST * NH * VW], BF16, name="v_sb")
            outT01 = stat.tile([128, S], BF16, name="outT01")
            outT23 = stat.tile([128, S], BF16, name="outT23")
            kb_sb = stat.tile([R, 2 * KC], BF16, name="kb_sb")
            ksh_sb = stat.tile([R, 2 * KC], BF16, name="ksh_sb")
            qb_sb = stat.tile([R, 2 * KC], BF16, name="qb_sb")
            qsh_sb = stat.tile([R, 2 * KC], BF16, name="qsh_sb")
            uv_sb = stat.tile([R, NH * HD], BF16, name="uv_sb")
            wo_sb = stat.tile([KC, 2 * D], BF16, name="wo_sb")
            eps_sb = stat.tile([128, 1], F32, name="eps_sb")

            cosP_sb = stat.tile([128, S], BF16, name="cosP_sb")
            sinP_sb = stat.tile([128, S], BF16, name="sinP_sb")
            tri_sb = stat.tile([128, 128], BF16, name="tri_sb")
            onesf_sb = stat.tile([128, 64], F32, name="onesf_sb")
            nc.gpsimd.memset(onesf_sb[:], 1.0)
            nc.gpsimd.memset(eps_sb[:], EPS)
            # tri[p, f] = 1.0 if p <= f else 0.0 (keep-lower-triangle gate
            # for diagonal score strips in k-major layout)
            nc.gpsimd.memset(tri_sb[:], 1.0)
            nc.gpsimd.affine_select(
                out=tri_sb[:], in_=tri_sb[:], compare_op=ALU.is_ge,
                fill=0.0, base=0, channel_multiplier=-1, pattern=[[1, 128]])
            # HAM pre-warm: ~7us of discarded matmuls during the initial
            # DMA wait so the first real matmul runs at 2.4GHz, not the
            # cold 1.2GHz (PE is otherwise idle until the x data lands)
            with tc.tile_pool(name="warm", bufs=1, space="PSUM") as wps:
                wt = wps.tile([128, SB], F32, name="wt")
                for _ in range(24):
                    nc.tensor.matmul(wt[:], tri_sb[:], ckvT[:, 0:SB])

            # ones column of v (col 64 of each 65-wide block)
            v_blocks = v_sb.rearrange("p (t h w) -> p t h w", t=NST, h=NH)
            nc.vector.tensor_copy(
                v_blocks[:, :, :, HD:VW],
                onesf_sb.rearrange("p (t h w) -> p t h w", t=NST, h=NH))

            xTr = xT.rearrange("(a p) s -> p a s", p=KC)   # [128, NKC, S]

            # ------- Phase 1+2 fused per s-block: down-proj + RMS norm +
            # ------- up-projections + rope + v  (keeps the PE warm) -------
            with tc.tile_pool(name="p1w", bufs=1) as p1w, \
                 tc.tile_pool(name="p1x", bufs=2) as p1x, \
                 tc.tile_pool(name="p1n", bufs=6) as p1n, \
                 tc.tile_pool(name="p2tmp", bufs=6) as p2tmp, \
                 tc.tile_pool(name="p1ps", bufs=2, space="PSUM") as p1ps, \
                 tc.tile_pool(name="p2ps", bufs=2, space="PSUM") as p2ps, \
                 tc.tile_pool(name="p2vps", bufs=2, space="PSUM") as p2vps:
                wkv_sb = p1w.tile([KC, D], BF16, name="wkv_sb")
                wq_sb = p1w.tile([KC, D], BF16, name="wq_sb")
                # critical-path DMA order: first chunk of the down-proj
                # weights, then x groups interleaved with the remaining
                # weight chunks (the k-loop only needs chunk g at step 4g),
                # then the small static weights.
                nc.sync.dma_start(wkv_sb[:, 0:512], wkv[:, 0:512])
                nc.sync.dma_start(wq_sb[:, 0:512], wq[:, 0:512])
                from concourse import bass_isa

                # rope for block sb runs one iteration later (software
                # pipeline) so its PE matmuls never head-of-line block the
                # next block's down-projection matmuls on the norm chain.
                def rope_block(sb):
                    sl = slice(sb * SB, (sb + 1) * SB)
                    # pair tensors: rows [he_nope|he_rope|ho_nope|ho_rope];
                    # cosP rows are 1.0 (sinP rows 0.0) on nope rows so one
                    # fused 3-op rope pass covers nope+rope together.
                    for cT, wb, wsh, dst in (
                            (ckvT, kb_sb, ksh_sb, (kT01, kT23)),
                            (cqT, qb_sb, qsh_sb, (qT01, qT23))):
                        for p in range(2):
                            ps2 = p2ps.tile([128, 2 * SB], F32, name="ps2",
                                            tag="p2")
                            nc.tensor.matmul(ps2[:, 0:SB],
                                             wb[:, p * KC:(p + 1) * KC],
                                             cT[:, sl])
                            nc.tensor.matmul(ps2[:, SB:2 * SB],
                                             wsh[:, p * KC:(p + 1) * KC],
                                             cT[:, sl])
                            t1 = p2tmp.tile([128, SB], BF16, name="t1", tag="t")
                            t2 = p2tmp.tile([128, SB], BF16, name="t2", tag="t")
                            nc.vector.tensor_mul(t1[:], ps2[:, 0:SB],
                                                 cosP_sb[:, sl])
                            nc.vector.tensor_mul(t2[:], ps2[:, SB:2 * SB],
                                                 sinP_sb[:, sl])
                            nc.vector.tensor_add(dst[p][:, sl], t1[:], t2[:])
                    # v tiles for this s-block (4 t-tiles)
                    for t in range(4 * sb, 4 * sb + 4):
                        vps = p2vps.tile([128, NH * HD], F32, name="vps",
                                         tag="v")
                        nc.tensor.matmul(vps[:], ckvT[:, t * ST:(t + 1) * ST],
                                         uv_sb[:])
                        nc.scalar.copy(
                            v_blocks[:, t, :, 0:HD],
                            vps.rearrange("p (h d) -> p h d", h=NH))

                for sb in range(NSB):
                    sl = slice(sb * SB, (sb + 1) * SB)
                    xt = p1x.tile([128, NKC * SB], BF16, name="xt", tag="xt")
                    xtv = xt.rearrange("p (a s) -> p a s", a=NKC)
                    for g in range(4):
                        nc.sync.dma_start(xtv[:, 4 * g:4 * g + 4, :],
                                          xTr[:, 4 * g:4 * g + 4, sl])
                        if sb == 0 and g < 3:
                            csl = slice((g + 1) * 512, (g + 2) * 512)
                            nc.sync.dma_start(wkv_sb[:, csl], wkv[:, csl])
                            nc.sync.dma_start(wq_sb[:, csl], wq[:, csl])
                    # deferred static loads, slotted behind the x blocks
                    nc.sync.dma_start(cosP_sb[:, sl], cosP[:, sl])
                    nc.sync.dma_start(sinP_sb[:, sl], sinP[:, sl])
                    if sb == 0:
                        nc.sync.dma_start(kb_sb[:], kb)
                        nc.sync.dma_start(ksh_sb[:], ksh)
                        nc.sync.dma_start(qb_sb[:], qb)
                        nc.sync.dma_start(qsh_sb[:], qsh)
                        nc.sync.dma_start(uv_sb[:], uv)
                    elif sb == 1:
                        nc.sync.dma_start(wo_sb[:], wo)
                    cps = {nm: p1ps.tile([128, SB], F32, name=f"cps_{nm}",
                                         tag="cps") for nm in ("kv", "q")}
                    for k in range(NKC):
                        for nm, wsb in (("kv", wkv_sb), ("q", wq_sb)):
                            nc.tensor.matmul(
                                cps[nm][:],
                                wsb[:, k * KC:(k + 1) * KC],
                                xtv[:, k, :],
                                start=(k == 0), stop=(k == NKC - 1))
                    # both PSUM->SBUF copies first: frees the cps banks for
                    # the next block's matmuls as early as possible
                    for nm, cT in (("kv", ckvT), ("q", cqT)):
                        nc.scalar.copy(cT[:, sl], cps[nm][:])
                    for nm, cT in (("kv", ckvT), ("q", cqT)):
                        sqt = p1n.tile([128, SB], F32, name="sqt", tag="sqt")
                        rst = p1n.tile([128, SB], F32, name="rst", tag="rst")
                        nc.vector.tensor_mul(sqt[:], cT[:, sl], cT[:, sl])
                        nc.gpsimd.partition_all_reduce(
                            rst[:], sqt[:], channels=128,
                            reduce_op=bass_isa.ReduceOp.add)
                        # rstd = (sum/R + eps)^-1/2 = exp(-0.5 * ln(...))
                        nc.scalar.activation(rst[:], rst[:], AF.Ln,
                                             bias=eps_sb[:], scale=1.0 / R)
                        nc.scalar.activation(rst[:], rst[:], AF.Exp,
                                             scale=-0.5)
                        nc.vector.tensor_mul(cT[:, sl], cT[:, sl], rst[:])
                    if sb > 1:
                        rope_block(sb - 2)
                rope_block(NSB - 2)
                rope_block(NSB - 1)

            if debug_dump:
                nc.sync.dma_start(dbg["d_ckvT"], ckvT[:])
                nc.sync.dma_start(dbg["d_cqT"], cqT[:])
                nc.sync.dma_start(dbg["d_kT01"], kT01[:])
                nc.sync.dma_start(dbg["d_kT23"], kT23[:])
                nc.sync.dma_start(dbg["d_qT01"], qT01[:])
                nc.sync.dma_start(dbg["d_qT23"], qT23[:])
                nc.sync.dma_start(dbg["d_v"], v_sb[:])

            # ------- Phase 3+4: attention + interleaved output proj ------
            with tc.tile_pool(name="p3e", bufs=4) as p3e, \
                 tc.tile_pool(name="p3rc", bufs=4) as p3rc, \
                 tc.tile_pool(name="p3rb", bufs=4) as p3rb, \
                 tc.tile_pool(name="p4y", bufs=4) as p4y, \
                 tc.tile_pool(name="p3sc", bufs=2, space="PSUM") as p3sc, \
                 tc.tile_pool(name="p3oa", bufs=2, space="PSUM") as p3oa, \
                 tc.tile_pool(name="p4ps", bufs=2, space="PSUM") as p4ps:
                # output projection deferred one j-block so its matmuls
                # never wait on the softmax eviction chain
                def p4_block(j):
                    for t in range(4 * j, 4 * j + 4):
                        for db in range(NSB):
                            dsl = slice(db * SB, (db + 1) * SB)
                            yps = p4ps.tile([128, SB], F32, name="yps",
                                            tag="yp")
                            for c, oT in ((0, outT01), (1, outT23)):
                                nc.tensor.matmul(
                                    yps[:],
                                    oT[:, t * ST:(t + 1) * ST],
                                    wo_sb[:, c * D + db * SB:
                                          c * D + (db + 1) * SB],
                                    start=(c == 0), stop=(c == 1))
                            ysb = p4y.tile([128, SB], BF16, name="ysb",
                                           tag="y")
                            nc.vector.tensor_copy(ysb[:], yps[:])
                            nc.sync.dma_start(
                                y[t * ST:(t + 1) * ST, dsl], ysb[:])

                for j in range(NSB):
                    for hp in range(2):      # head pairs (0,1) and (2,3)
                        kTp = kT01 if hp == 0 else kT23
                        qTp = qT01 if hp == 0 else qT23
                        oa = [p3oa.tile([VW, SB], F32, name=f"oa{j}_{hp}{u}",
                                        tag="oa") for u in range(2)]
                        ktiles = list(range(4 * j + 4))
                        pend = None   # (i, q0, et) awaiting its av matmuls
                        for i in ktiles + [None]:
                            if i is not None:
                                q0 = 128 * (i - 4 * j) if i >= 4 * j else 0
                                scp = p3sc.tile([128, 2 * SB], F32,
                                                name="scp", tag="sc")
                                for u in range(2):
                                    hs = slice(u * 64, u * 64 + 64)
                                    nc.tensor.matmul(
                                        scp[:, u * SB + q0:(u + 1) * SB],
                                        kTp[hs, i * 128:(i + 1) * 128],
                                        qTp[hs, j * SB + q0:(j + 1) * SB])
                                et = p3e.tile([128, 2 * SB], BF16, name="et",
                                              tag="et")
                                if q0 == 0:
                                    nc.scalar.activation(et[:], scp[:],
                                                         AF.Exp, scale=0.125)
                                else:
                                    sc3 = scp.rearrange("p (g s) -> p g s",
                                                        g=2)
                                    et3 = et.rearrange("p (g s) -> p g s",
                                                       g=2)
                                    nc.scalar.activation(
                                        et3[:, :, q0:SB], sc3[:, :, q0:SB],
                                        AF.Exp, scale=0.125)
                                if i >= 4 * j:
                                    for u in range(2):
                                        nc.vector.tensor_mul(
                                            et[:, u * SB + q0:
                                               u * SB + q0 + 128],
                                            et[:, u * SB + q0:
                                               u * SB + q0 + 128],
                                            tri_sb[:])
                                if debug_dump and j == 0 and hp == 0 and i == 0:
                                    nc.sync.dma_start(dbg["d_et"], et[:])
                            # av matmuls for the previous i-tile, emitted
                            # after the next scores so the PE never idles
                            # waiting on the exp chain
                            if pend is not None:
                                pi, pq0, pet = pend
                                for u in range(2):
                                    h = 2 * hp + u
                                    nc.tensor.matmul(
                                        oa[u][:, pq0:SB],
                                        v_sb[:, pi * (NH * VW) + h * VW:
                                             pi * (NH * VW) + (h + 1) * VW],
                                        pet[:, u * SB + pq0:(u + 1) * SB],
                                        start=(pi == 0),
                                        stop=(pi == ktiles[-1]))
                            pend = (i, q0, et) if i is not None else None
                        for u in range(2):
                            h = 2 * hp + u
                            # evacuate oacc to SBUF in one fast copy so the
                            # PSUM bank recycles immediately; the normalize
                            # chain then runs entirely off SBUF
                            oc = p3rb.tile([VW, SB], F32, name="oc", tag="oc")
                            nc.vector.tensor_copy(oc[:], oa[u][:])
                            rc = p3rc.tile([1, SB], F32, name="rc", tag="rc")
                            nc.scalar.activation(rc[:], oc[HD:VW, :], AF.Ln)
                            nc.scalar.activation(rc[:], rc[:], AF.Exp,
                                                 scale=-1.0)
                            rb = p3rb.tile([HD, SB], F32, name="rb", tag="rb")
                            nc.gpsimd.partition_broadcast(rb[:], rc[:])
                            dstT = (outT01 if h < 2 else outT23)[
                                (h % 2) * HD:(h % 2 + 1) * HD,
                                j * SB:(j + 1) * SB]
                            nc.vector.tensor_mul(dstT, oc[0:HD, :], rb[:])
                    if j > 0:
                        p4_block(j - 1)
                p4_block(NSB - 1)
                if debug_dump:
                    nc.sync.dma_start(dbg["d_outT01"], outT01[:])
                    nc.sync.dma_start(dbg["d_outT23"], outT23[:])

    nc.finalize()
    return nc


def _build_nc_generic(causal: bool, use_mask: bool):
    """Baseline builder (kept as fallback for non-causal / masked inputs)."""
    nc = bacc.Bacc("TRN2", target_bir_lowering=False, debug=False,
                   num_devices=NCORES)

    xT = nc.dram_tensor("xT", [D, S], BF16, kind="ExternalInput").ap()
    wkv = nc.dram_tensor("wkv", [KC, D], BF16, kind="ExternalInput").ap()
    wq = nc.dram_tensor("wq", [KC, D], BF16, kind="ExternalInput").ap()
    kb = nc.dram_tensor("kb", [R, 2 * KC], BF16, kind="ExternalInput").ap()
    ksh = nc.dram_tensor("ksh", [R, 2 * KC], BF16, kind="ExternalInput").ap()
    qb = nc.dram_tensor("qb", [R, 2 * KC], BF16, kind="ExternalInput").ap()
    qsh = nc.dram_tensor("qsh", [R, 2 * KC], BF16, kind="ExternalInput").ap()
    uv = nc.dram_tensor("uv", [R, NH * HD], BF16, kind="ExternalInput").ap()
    wo = nc.dram_tensor("wo", [KC, 2 * D], BF16, kind="ExternalInput").ap()
    cosP = nc.dram_tensor("cosP", [128, S], F32, kind="ExternalInput").ap()
    sinP = nc.dram_tensor("sinP", [128, S], F32, kind="ExternalInput").ap()
    maskT = None
    if use_mask:
        maskT = nc.dram_tensor("maskT", [S, S], F32, kind="ExternalInput").ap()
    y = nc.dram_tensor("y", [S, D], F32, kind="ExternalOutput").ap()

    AF = mybir.ActivationFunctionType
    ALU = mybir.AluOpType

    with tile.TileContext(nc) as tc:
        from contextlib import ExitStack
        with ExitStack() as ctx:
            stat = ctx.enter_context(tc.tile_pool(name="static", bufs=1))
            ckvT = stat.tile([R, S], BF16, name="ckvT")
            cqT = stat.tile([R, S], BF16, name="cqT")
            kT01 = stat.tile([128, S], BF16, name="kT01")
            kT23 = stat.tile([128, S], BF16, name="kT23")
            qT01 = stat.tile([128, S], BF16, name="qT01")
            qT23 = stat.tile([128, S], BF16, name="qT23")
            v_sb = stat.tile([128, NST * NH * VW], BF16, name="v_sb")
            outT01 = stat.tile([128, S], BF16, name="outT01")
            outT23 = stat.tile([128, S], BF16, name="outT23")
            kb_sb = stat.tile([R, 2 * KC], BF16, name="kb_sb")
            ksh_sb = stat.tile([R, 2 * KC], BF16, name="ksh_sb")
            qb_sb = stat.tile([R, 2 * KC], BF16, name="qb_sb")
            qsh_sb = stat.tile([R, 2 * KC], BF16, name="qsh_sb")
            uv_sb = stat.tile([R, NH * HD], BF16, name="uv_sb")
            wo_sb = stat.tile([KC, 2 * D], BF16, name="wo_sb")
            eps_sb = stat.tile([128, 1], F32, name="eps_sb")

            nc.sync.dma_start(kb_sb[:], kb)
            nc.sync.dma_start(ksh_sb[:], ksh)
            nc.sync.dma_start(qb_sb[:], qb)
            nc.sync.dma_start(qsh_sb[:], qsh)
            nc.sync.dma_start(uv_sb[:], uv)
            nc.sync.dma_start(wo_sb[:], wo)
            tri_sb = stat.tile([128, 128], BF16, name="tri_sb")
            onesf_sb = stat.tile([128, 64], F32, name="onesf_sb")
            nc.gpsimd.memset(onesf_sb[:], 1.0)
            nc.gpsimd.memset(eps_sb[:], EPS)
            nc.gpsimd.memset(tri_sb[:], 1.0)
            nc.gpsimd.affine_select(
                out=tri_sb[:], in_=tri_sb[:], compare_op=ALU.is_ge,
                fill=0.0, base=0, channel_multiplier=-1, pattern=[[1, 128]])
            v_blocks = v_sb.rearrange("p (t h w) -> p t h w", t=NST, h=NH)
            nc.vector.tensor_copy(
                v_blocks[:, :, :, HD:VW],
                onesf_sb.rearrange("p (t h w) -> p t h w", t=NST, h=NH))

            from concourse import bass_isa
            with tc.tile_pool(name="p1w", bufs=1) as p1w, \
                 tc.tile_pool(name="p1n", bufs=8) as p1n, \
                 tc.tile_pool(name="p1x", bufs=8) as p1x, \
                 tc.tile_pool(name="p1ps", bufs=6, space="PSUM") as p1ps:
                wkv_sb = p1w.tile([KC, D], BF16, name="wkv_sb")
                wq_sb = p1w.tile([KC, D], BF16, name="wq_sb")
                nc.sync.dma_start(wkv_sb[:], wkv)
                nc.sync.dma_start(wq_sb[:], wq)

                for sb in range(NSB):
                    sl = slice(sb * SB, (sb + 1) * SB)
                    cps = {nm: p1ps.tile([128, SB], F32, name=f"cps_{nm}",
                                         tag="cps") for nm in ("kv", "q")}
                    for k in range(NKC):
                        xt = p1x.tile([128, SB], BF16, name="xt", tag="xt")
                        nc.sync.dma_start(xt[:], xT[k * KC:(k + 1) * KC, sl])
                        for nm, wsb in (("kv", wkv_sb), ("q", wq_sb)):
                            nc.tensor.matmul(
                                cps[nm][:],
                                wsb[:, k * KC:(k + 1) * KC],
                                xt[:],
                                start=(k == 0), stop=(k == NKC - 1))
                    for nm, cT in (("kv", ckvT), ("q", cqT)):
                        sqt = p1n.tile([128, SB], F32, name="sqt", tag="sqt")
                        rst = p1n.tile([128, SB], F32, name="rst", tag="sqt")
                        nc.vector.tensor_copy(cT[:, sl], cps[nm][:])
                        nc.vector.tensor_mul(sqt[:], cT[:, sl], cT[:, sl])
                        nc.gpsimd.partition_all_reduce(
                            rst[:], sqt[:], channels=128,
                            reduce_op=bass_isa.ReduceOp.add)
                        nc.scalar.activation(rst[:], rst[:], AF.Ln,
                                             bias=eps_sb[:], scale=1.0 / R)
                        nc.scalar.activation(rst[:], rst[:], AF.Exp,
                                             scale=-0.5)
                        nc.vector.tensor_mul(cT[:, sl], cT[:, sl], rst[:])

            with tc.tile_pool(name="p2t", bufs=1) as p2t, \
                 tc.tile_pool(name="p2tmp", bufs=6) as p2tmp, \
                 tc.tile_pool(name="p2ps", bufs=6, space="PSUM") as p2ps, \
                 tc.tile_pool(name="p2vps", bufs=2, space="PSUM") as p2vps:
                cosP_sb = p2t.tile([128, S], F32, name="cosP_sb")
                sinP_sb = p2t.tile([128, S], F32, name="sinP_sb")
                nc.sync.dma_start(cosP_sb[:], cosP)
                nc.sync.dma_start(sinP_sb[:], sinP)

                for sb in range(NSB):
                    sl = slice(sb * SB, (sb + 1) * SB)
                    for cT, wb, wsh, dst in (
                            (ckvT, kb_sb, ksh_sb, (kT01, kT23)),
                            (cqT, qb_sb, qsh_sb, (qT01, qT23))):
                        for p in range(2):
                            pb = p2ps.tile([128, SB], F32, name="pb", tag="p2")
                            psh = p2ps.tile([128, SB], F32, name="psh", tag="p2")
                            nc.tensor.matmul(pb[:], wb[:, p * KC:(p + 1) * KC],
                                             cT[:, sl])
                            nc.tensor.matmul(psh[:], wsh[:, p * KC:(p + 1) * KC],
                                             cT[:, sl])
                            t1 = p2tmp.tile([128, SB], F32, name="t1", tag="t")
                            t2 = p2tmp.tile([128, SB], F32, name="t2", tag="t")
                            nc.vector.tensor_mul(t1[:], pb[:], cosP_sb[:, sl])
                            nc.vector.tensor_mul(t2[:], psh[:], sinP_sb[:, sl])
                            nc.vector.tensor_add(dst[p][:, sl], t1[:], t2[:])

                for t in range(NST):
                    vps = p2vps.tile([128, NH * HD], F32, name="vps", tag="v")
                    nc.tensor.matmul(vps[:], (ckvT[:, t * ST:(t + 1) * ST]),
                                     (uv_sb[:]))
                    dst = v_blocks[:, t, :, 0:HD]
                    src = vps.rearrange("p (h d) -> p h d", h=NH)
                    nc.vector.tensor_copy(dst, src)

            with tc.tile_pool(name="p3e", bufs=8) as p3e, \
                 tc.tile_pool(name="p3m", bufs=3) as p3m, \
                 tc.tile_pool(name="p3rc", bufs=8) as p3rc, \
                 tc.tile_pool(name="p3rb", bufs=6) as p3rb, \
                 tc.tile_pool(name="p3sc", bufs=4, space="PSUM") as p3sc, \
                 tc.tile_pool(name="p3oa", bufs=4, space="PSUM") as p3oa:
                for j in range(NSB):
                    ktiles = list(range(4 * j + 4)) if causal else list(range(NST))
                    oacc = [p3oa.tile([VW, SB], F32, name=f"oa{j}_{h}", tag="oa")
                            for h in range(NH)]
                    for i in ktiles:
                        q0 = 128 * (i - 4 * j) if (causal and i >= 4 * j) else 0
                        qsl = slice(q0, SB)
                        mt = None
                        if use_mask:
                            mt = p3m.tile([128, SB], F32, name="mt", tag="mt")
                            nc.sync.dma_start(
                                mt[:], maskT[i * 128:(i + 1) * 128,
                                             j * SB:(j + 1) * SB])
                        for h in range(NH):
                            kTp = kT01 if h < 2 else kT23
                            qTp = qT01 if h < 2 else qT23
                            hs = slice((h % 2) * 64, (h % 2) * 64 + 64)
                            sc = p3sc.tile([128, SB], F32, name="sc", tag="sc")
                            nc.tensor.matmul(
                                sc[:, qsl],
                                kTp[hs, i * 128:(i + 1) * 128],
                                qTp[hs, j * SB + q0:(j + 1) * SB])
                            if use_mask:
                                nc.vector.tensor_add(sc[:, qsl], sc[:, qsl],
                                                     mt[:, qsl])
                            et = p3e.tile([128, SB], BF16, name="et", tag="e")
                            nc.scalar.activation(et[:, qsl], sc[:, qsl],
                                                 AF.Exp, scale=0.125)
                            if causal and i >= 4 * j:
                                nc.vector.tensor_mul(et[:, q0:q0 + 128],
                                                     et[:, q0:q0 + 128],
                                                     tri_sb[:])
                            nc.tensor.matmul(
                                oacc[h][:, qsl],
                                (v_sb[:, i * (NH * VW) + h * VW:
                                        i * (NH * VW) + (h + 1) * VW]),
                                (et[:, qsl]),
                                start=(i == ktiles[0]), stop=(i == ktiles[-1]))
                    for h in range(NH):
                        rc = p3rc.tile([1, SB], F32, name="rc", tag="rc")
                        nc.scalar.activation(rc[:], oacc[h][HD:VW, :], AF.Ln)
                        nc.scalar.activation(rc[:], rc[:], AF.Exp, scale=-1.0)
                        rb = p3rb.tile([HD, SB], F32, name="rb", tag="rb")
                        nc.gpsimd.partition_broadcast(rb[:], rc[:])
                        dst = (outT01 if h < 2 else outT23)[
                            (h % 2) * HD:(h % 2 + 1) * HD,
                            j * SB:(j + 1) * SB]
                        nc.vector.tensor_mul(dst, oacc[h][0:HD, :], rb[:])

            with tc.tile_pool(name="p4y", bufs=4) as p4y, \
                 tc.tile_pool(name="p4ps", bufs=2, space="PSUM") as p4ps:
                for t in range(NST):
                    yp = p4ps.tile([128, S], F32, name="yp", tag="yp")
                    for db in range(NSB):
                        dsl = slice(db * SB, (db + 1) * SB)
                        for c, oT in ((0, outT01), (1, outT23)):
                            nc.tensor.matmul(
                                yp[:, dsl],
                                (oT[:, t * ST:(t + 1) * ST]),
                                (wo_sb[:, c * D + db * SB:c * D + (db + 1) * SB]),
                                start=(c == 0), stop=(c == 1))
                    ysb = p4y.tile([128, S], F32, name="ysb", tag="y")
                    nc.vector.tensor_copy(ysb[:], yp[:])
                    nc.sync.dma_start(y[t * ST:(t + 1) * ST, :], ysb[:])

    nc.finalize()
    return nc


_NC_CACHE = {}


def _get_nc(causal, use_mask):
    key = (causal, use_mask)
    if key not in _NC_CACHE:
        if causal and not use_mask:
            _NC_CACHE[key] = _build_nc_fast()
        else:
            _NC_CACHE[key] = _build_nc_generic(causal, use_mask)
    return _NC_CACHE[key]


def _prep_inputs(x, cos, sin, mask, w_kv_down, kv_norm_w, w_uk, w_ur, w_uv,
                 w_q_down, q_norm_w, w_uq, w_qr, w_o, use_mask):
    """Build the 8 per-core input maps (host-side shard + fold)."""
    f = np.float32
    x = np.asarray(x, f)
    cos = np.asarray(cos, f)
    sin = np.asarray(sin, f)
    w_kv_down = np.asarray(w_kv_down, f)
    w_q_down = np.asarray(w_q_down, f)
    kv_norm_w = np.asarray(kv_norm_w, f)
    q_norm_w = np.asarray(q_norm_w, f)
    w_uk_e = np.asarray(w_uk, f) * kv_norm_w[:, None]
    w_ur_e = np.asarray(w_ur, f) * kv_norm_w[:, None]
    w_uv_e = np.asarray(w_uv, f) * kv_norm_w[:, None]
    w_uq_e = np.asarray(w_uq, f) * q_norm_w[:, None]
    w_qr_e = np.asarray(w_qr, f) * q_norm_w[:, None]
    w_o = np.asarray(w_o, f)

    # shared rearrangements
    wkv = np.ascontiguousarray(
        w_kv_down.reshape(NKC, KC, R).transpose(1, 0, 2).reshape(KC, D))
    wq = np.ascontiguousarray(
        w_q_down.reshape(NKC, KC, R).transpose(1, 0, 2).reshape(KC, D))
    cosT = np.ascontiguousarray(cos.T)                 # [32, S]
    sinT = np.ascontiguousarray(sin.T)
    sinSg = np.concatenate([-sinT[:DR // 2], sinT[DR // 2:]], axis=0)
    one32 = np.ones((DR, S), np.float32)
    zero32 = np.zeros((DR, S), np.float32)
    # pair-tensor rope tables: nope rows pass through (cos=1, sin=0)
    cosPt = np.ascontiguousarray(
        np.concatenate([one32, cosT, one32, cosT], axis=0))
    sinPt = np.ascontiguousarray(
        np.concatenate([zero32, sinSg, zero32, sinSg], axis=0))
    # rope shift permutation within each head's 32 cols
    perm = np.concatenate([np.arange(16, 32), np.arange(0, 16)])

    import ml_dtypes as _md
    xTb = [np.ascontiguousarray(x[b].T).astype(_md.bfloat16) for b in range(B)]
    maskT8 = None
    if use_mask:
        m = np.asarray(mask, f).reshape(S, S)
        maskT8 = np.ascontiguousarray(m.T) * 8.0

    in_maps = []
    z32 = np.zeros((R, DN), np.float32)
    for core in range(NCORES):
        b, g = core // 4, core % 4
        cs = slice(g * NH * DN, (g + 1) * NH * DN)      # 128-wide col slice
        vs = slice(g * NH * HD, (g + 1) * NH * HD)      # 256-wide
        uk_l = w_uk_e[:, cs].reshape(R, NH, DN)
        ur_l = w_ur_e[:, cs].reshape(R, NH, DR)
        urs_l = ur_l[:, :, perm]
        uq_l = w_uq_e[:, cs].reshape(R, NH, DN)
        qr_l = w_qr_e[:, cs].reshape(R, NH, DR)
        qrs_l = qr_l[:, :, perm]
        # pair layout: [he_nope | he_rope | ho_nope | ho_rope] per 128 cols
        def pair(nope, rope):
            cols = []
            for h in range(NH):
                cols += [nope[:, h], rope[:, h]]
            return np.ascontiguousarray(np.concatenate(cols, axis=1))
        def pair_sh(sh):
            cols = []
            for h in range(NH):
                cols += [z32, sh[:, h]]
            return np.ascontiguousarray(np.concatenate(cols, axis=1))
        wo_loc = w_o[g * NH * HD:(g + 1) * NH * HD]     # [256, D]
        wo_r = np.ascontiguousarray(
            wo_loc.reshape(2, KC, D).transpose(1, 0, 2).reshape(KC, 2 * D)
        ).astype(_md.bfloat16)
        cosW = cosPt if use_mask else cosPt.astype(_md.bfloat16)
        sinW = sinPt if use_mask else sinPt.astype(_md.bfloat16)
        m_ = {
            "xT": xTb[b],
            "wkv": wkv.astype(_md.bfloat16), "wq": wq.astype(_md.bfloat16),
            "kb": pair(uk_l, ur_l).astype(_md.bfloat16),
            "ksh": pair_sh(urs_l).astype(_md.bfloat16),
            "qb": pair(uq_l, qr_l).astype(_md.bfloat16),
            "qsh": pair_sh(qrs_l).astype(_md.bfloat16),
            "uv": np.ascontiguousarray(w_uv_e[:, vs]).astype(_md.bfloat16),
            "wo": wo_r,
            "cosP": cosW, "sinP": sinW,
        }
        if use_mask:
            m_["maskT"] = maskT8
        in_maps.append(m_)
    return in_maps


def _classify_mask(mask):
    m = np.asarray(mask, np.float32).reshape(S, S)
    if not np.any(m):
        return False, False          # dense, no mask
    causal_ref = np.where(
        np.tril(np.ones((S, S), dtype=bool)), np.float32(0.0),
        np.float32(-1e9))
    if np.array_equal(m, causal_ref):
        return True, False           # structural causal
    return False, True               # generic additive mask


LAST_RESULTS = None


def kernel(**inputs):
    global LAST_RESULTS
    from concourse.bass_utils import run_bass_kernel_spmd
    causal, use_mask = _classify_mask(inputs["mask"])
    nc = _get_nc(causal, use_mask)
    in_maps = _prep_inputs(
        inputs["x"], inputs["cos"], inputs["sin"], inputs["mask"],
        inputs["w_kv_down"], inputs["kv_norm_w"], inputs["w_uk"],
        inputs["w_ur"], inputs["w_uv"], inputs["w_q_down"],
        inputs["q_norm_w"], inputs["w_uq"], inputs["w_qr"], inputs["w_o"],
        use_mask)
    res = run_bass_kernel_spmd(nc, in_maps, list(range(NCORES)))
    LAST_RESULTS = res
    parts = [res.results[c]["y"] for c in range(NCORES)]
    out = np.empty((B, S, D), np.float32)
    for b in range(B):
        out[b] = (parts[4 * b].astype(np.float32)
                  + parts[4 * b + 1].astype(np.float32)
                  + parts[4 * b + 2].astype(np.float32)
                  + parts[4 * b + 3].astype(np.float32))
    return out
